# revision 28
# baseline (speedup 1.0000x reference)
"""CLIPVisionEmbeddings Trainium2 kernel.

Computes, for full inputs:
    x   = LayerNorm(patches, g, b)                 # [N, PD]
    pe  = x @ W_patch.T                            # [N, D]
    pos = bbox_coords @ W_pos.T + b_pos            # [N, D]
    out = concat([cls_embed[0] + cls_pos, pe + pos])[None]   # [1, N+1, D]

Strategy: shard the N=32768 patch axis across 8 NeuronCores (4096 rows
each), replicate the small weights. Per core, a Tile kernel does:
  - LN stats via bn_stats/bn_aggr in [n, p] layout (free-dim reduce)
  - normalize + bf16 cast in one DVE tensor_scalar with per-partition
    mean/rstd scalars
  - PE-transpose the normalized tile to [p, n] layout (two groups of 3,
    software-pipelined one tile ahead of the matmuls)
  - 6 K=128 matmuls against the pre-transposed (g-scaled) W, plus one
    K=5 matmul accumulating the bbox positional term AND the bias row
    (blhs row 4 = ones, wposT row 4 = W_patch @ ln1_b + b_pos) into the
    same PSUM accumulation group
  - PSUM -> SBUF copies on ACT, then contiguous DMA out

Host side only reshapes/shards: W transpose+scale, bbox transpose with
appended ones row, the single CLS output row, and the final gather.
"""

import os
import sys

import numpy as np

for _p in ("/opt/trn_rl_repo", "/root/.axon_site/_ro/trn_rl_repo"):
    if os.path.isdir(_p) and _p not in sys.path:
        sys.path.append(_p)

import ml_dtypes

N, PD, D = 32768, 768, 1024
NCORES = 8
RC = N // NCORES          # rows per core: 4096
NT = RC // 128            # n-tiles per core: 32
KT = PD // 128            # k-tiles: 6
EPS = 1e-5

# matmul input dtype: "bf16" or "fp32r"
MM_DTYPE = os.environ.get("KERNEL_MM_DTYPE", "bf16")

_CACHE = {}


def _build_nc(repeat=None):
    import concourse.bass as bass
    import concourse.tile as tile
    from concourse import bacc, mybir
    from concourse.bass import ts
    from concourse.masks import make_identity

    mdt = mybir.dt.bfloat16 if MM_DTYPE == "bf16" else mybir.dt.float32r
    f32 = mybir.dt.float32

    nc = bacc.Bacc("TRN2", target_bir_lowering=False, debug=False)

    x = nc.dram_tensor("x", [RC, PD], mdt, kind="ExternalInput").ap()
    wt = nc.dram_tensor("wt", [PD, D], mdt, kind="ExternalInput").ap()
    # blhs rows: 0-3 bbox.T, 4 ones (bias row carrier)
    blhs = nc.dram_tensor("blhs", [5, RC], mdt, kind="ExternalInput").ap()
    # wposT rows: 0-3 W_pos.T, 4 crow (= W_patch @ ln1_b + b_pos)
    wposT = nc.dram_tensor("wposT", [5, D], mdt, kind="ExternalInput").ap()
    y = nc.dram_tensor("y", [RC, D], f32, kind="ExternalOutput").ap()

    from contextlib import ExitStack

    def _b(name, default):
        return int(os.environ.get(f"KB_{name}", default))

    with tile.TileContext(nc) as tc, ExitStack() as ctx:
        consts = ctx.enter_context(tc.tile_pool(name="consts", bufs=1))
        xp = ctx.enter_context(tc.tile_pool(name="xp", bufs=_b("xp", 4)))
        sp = ctx.enter_context(tc.tile_pool(name="sp", bufs=_b("sp", 4)))
        hp = ctx.enter_context(tc.tile_pool(name="hp", bufs=_b("hp", 4)))
        tp = ctx.enter_context(tc.tile_pool(name="tp", bufs=_b("tp", 4), space="PSUM"))
        mp = ctx.enter_context(tc.tile_pool(name="mp", bufs=_b("mp", 2), space="PSUM"))
        op = ctx.enter_context(tc.tile_pool(name="op", bufs=_b("op", 3)))

        # --- prefetch first x tiles before the bulky weight DMA ---
        # (skipped in repeat/timing mode so every body iteration is identical)
        prefetched = {}
        if repeat is None:
            for i in range(2):
                xt = xp.tile([128, PD], mdt, tag="xt")
                nc.sync.dma_start(out=xt, in_=x[ts(i, 128), :])
                prefetched[i] = xt

        # --- constants ---
        wt_r = wt.rearrange("(k p) d -> p k d", p=128)
        wt_sb = consts.tile([128, KT, D], mdt)
        nc.sync.dma_start(out=wt_sb[:, 0, :], in_=wt_r[:, 0, :])
        blhs_sb = consts.tile([5, RC], mdt)
        nc.sync.dma_start(out=blhs_sb, in_=blhs)
        wposT_sb = consts.tile([5, D], mdt)
        nc.sync.dma_start(out=wposT_sb, in_=wposT)
        ident = consts.tile([128, 128], mdt)
        make_identity(nc, ident)
        eps_sb = consts.tile([128, 1], f32)
        nc.vector.memset(eps_sb, EPS)
        for k in range(1, KT):
            nc.sync.dma_start(out=wt_sb[:, k, :], in_=wt_r[:, k, :])

        def emit_ln(i):
            if i in prefetched:
                xt = prefetched.pop(i)
            else:
                xt = xp.tile([128, PD], mdt, tag="xt")
                nc.sync.dma_start(out=xt, in_=x[ts(i, 128), :])

            # LN stats: mean/var over the free (feature) dim
            stats = sp.tile([128, 2, 6], f32)
            xg = xt.rearrange("p (s f) -> p s f", s=2)
            for s in range(2):
                nc.vector.bn_stats(out=stats[:, s, :], in_=xg[:, s, :])
            mv = sp.tile([128, 2], f32)
            nc.vector.bn_aggr(out=mv, in_=stats)

            std = sp.tile([128, 1], f32)
            nc.scalar.activation(out=std, in_=mv[:, 1:2],
                                 func=mybir.ActivationFunctionType.Sqrt,
                                 bias=eps_sb)
            rstd = sp.tile([128, 1], f32)
            nc.vector.reciprocal(out=rstd, in_=std)

            # xhat = (x - mean) * rstd, cast to matmul dtype (DVE)
            xh = hp.tile([128, PD], mdt)
            nc.vector.tensor_scalar(out=xh, in0=xt, scalar1=mv[:, 0:1],
                                    scalar2=rstd, op0=mybir.AluOpType.subtract,
                                    op1=mybir.AluOpType.mult)
            return xh

        # timing-ablation switches; only honored in repeat/timing builds so
        # the graded single-shot build can never be affected
        skips = set()
        if repeat is not None:
            skips = set(os.environ.get("KERNEL_SKIP", "").split(","))

        def emit_transpose(xh):
            if "transpose" in skips:  # timing-ablation only (wrong results)
                return [wt_sb[:, 0, 0:384].rearrange("p (k n) -> p k n", k=3)] * 2
            # transpose to [p, n] layout, two pipelined groups of 3
            xhTs = []
            for g in range(2):
                pt = tp.tile([128, 3, 128], mdt, tag="pt")
                for k in range(3):
                    nc.tensor.transpose(out=pt[:, k, :],
                                        in_=xh[:, ts(3 * g + k, 128)],
                                        identity=ident)
                xhT = hp.tile([128, 3, 128], mdt, tag="xhT")
                if g == 0:
                    nc.scalar.copy(out=xhT, in_=pt)
                else:
                    nc.vector.tensor_copy(out=xhT, in_=pt)
                xhTs.append(xhT)
            return xhTs

        def emit_mm(i, xhTs):
            # matmuls: y[n, d] = sum_k xhT_k.T @ wt_k + blhs_i.T @ wposT
            # (wposT row 4 = bias row, blhs row 4 = ones)
            ps0 = mp.tile([128, 512], f32)
            ps1 = mp.tile([128, 512], f32)
            last_main = "bbox" in skips
            for k in range(KT):
                src = xhTs[k // 3][:, k % 3, :]
                nc.tensor.matmul(ps0, lhsT=src, rhs=wt_sb[:, k, 0:512],
                                 start=(k == 0), stop=last_main and k == KT - 1)
                nc.tensor.matmul(ps1, lhsT=src, rhs=wt_sb[:, k, 512:D],
                                 start=(k == 0), stop=last_main and k == KT - 1)
            if "bbox" in skips:  # timing-ablation only (wrong results)
                pass
            else:
                nc.tensor.matmul(ps0, lhsT=blhs_sb[:, ts(i, 128)],
                                 rhs=wposT_sb[:, 0:512], start=False, stop=True)
                nc.tensor.matmul(ps1, lhsT=blhs_sb[:, ts(i, 128)],
                                 rhs=wposT_sb[:, 512:D], start=False, stop=True)

            ot = op.tile([128, D], f32)
            nc.scalar.copy(out=ot[:, 0:512], in_=ps0)
            nc.scalar.copy(out=ot[:, 512:D], in_=ps1)
            nc.sync.dma_start(out=y[ts(i, 128), :], in_=ot)

        def body(rep):
            # 1-deep software pipeline: transposes for tile i+1 are emitted
            # before the matmuls of tile i so the PSUM->SBUF copies always
            # complete under the previous tile's matmul window.
            pend = None  # (i, xhTs)
            for i in range(NT):
                xh = emit_ln(i)
                xhTs = emit_transpose(xh)
                if pend is not None:
                    emit_mm(*pend)
                pend = (i, xhTs)
            emit_mm(*pend)

        if repeat is None:
            body(0)
        else:
            hints = (mybir.EngineType.PE, mybir.EngineType.DVE,
                     mybir.EngineType.Activation, mybir.EngineType.SP)
            with tc.For_i(0, repeat, 1, hint_engines=hints):
                body(0)

    nc.compile()
    return nc


def _host_prep(patches, bbox_coords, ln1_g, ln1_b, W_patch, cls_embed,
               W_pos, b_pos, cls_pos):
    np_mdt = ml_dtypes.bfloat16 if MM_DTYPE == "bf16" else np.float32
    Wg = W_patch.astype(np.float32) * ln1_g.astype(np.float32)[None, :]
    wt = np.ascontiguousarray(Wg.T).astype(np_mdt)                  # [PD, D]
    crow = (W_patch.astype(np.float64) @ ln1_b.astype(np.float64)
            + b_pos.astype(np.float64)).astype(np.float32)          # [D]
    wposT = np.empty((5, D), np.float32)
    wposT[0:4] = W_pos.astype(np.float32).T
    wposT[4] = crow
    wposT = wposT.astype(np_mdt)
    cls_row = (cls_embed[0, 0].astype(np.float32)
               + cls_pos[0].astype(np.float32))                      # [D]

    in_maps = []
    for c in range(NCORES):
        sl = slice(c * RC, (c + 1) * RC)
        blhs = np.empty((5, RC), np.float32)
        blhs[0:4] = bbox_coords[sl].T
        blhs[4] = 1.0
        in_maps.append({
            "x": np.ascontiguousarray(patches[sl]).astype(np_mdt),
            "wt": wt,
            "blhs": blhs.astype(np_mdt),
            "wposT": wposT,
        })
    return in_maps, cls_row


def get_nc(repeat=None):
    key = ("nc", repeat)
    if key not in _CACHE:
        _CACHE[key] = _build_nc(repeat)
    return _CACHE[key]


def kernel(**inputs):
    from concourse import bass_utils

    inputs = {k: np.asarray(v) for k, v in inputs.items()}
    in_maps, cls_row = _host_prep(**inputs)
    nc = get_nc()
    res = bass_utils.run_bass_kernel_spmd(nc, in_maps, core_ids=list(range(NCORES)))
    out = np.empty((1, N + 1, D), np.float32)
    out[0, 0] = cls_row
    for c in range(NCORES):
        out[0, 1 + c * RC: 1 + (c + 1) * RC] = res.results[c]["y"]
    return out
